# revision 38
# baseline (speedup 1.0000x reference)
"""Trainium2 Bass kernel for nn_MHA_65429531787938.

MHA with a faithful-quirk softmax over dim=0 (the batch axis, B=2).
For B=2 the batch-softmax collapses to an elementwise sigmoid:
    attn0 = sigmoid((s0 - s1)/SCALE),  attn1 = 1 - attn0
and (1-A0) @ V1 = colsum(V1) - A0 @ V1, so a single attention matrix
serves both batches.

Sharding: tensor-parallel over the 16 heads -> 2 heads per core
(columns of w_q/w_k/w_v, rows of W_o). Each core consumes the full x
and produces a partial output (its heads' contribution to out = vals @ W_o);
the host sums the 8 partials.

Per-core pipeline (heads h0=2i, h1=2i+1 -> a 128-wide slice of q/k/v dims):
  phase 1: x cast to fp16 on load -> xT via PE transpose; qT,kT,vT
           projections (N=512 fp16 matmuls, fp32 psum);
           qT/kT stored batch-stacked per head ([Q0;-Q1] / [K0;K1]);
           vT -> V natural via PE transpose (V1 stored negated).
  phase 2: d^T = K0@Q0^T - K1@Q1^T in one fused matmul (contraction=128);
           A0^T = sigmoid(d^T/SCALE) on ACT (fp16 out);
           psum_av = [V0 | -V1] @ A0^T + rank-1 colsum(V1) correction.
  phase 3: out_partial = vals @ W_o_slice (fp16 operands, fp32 psum).

Precision: fp16 operands everywhere, fp32 accumulation; measured
end-to-end rel err ~3e-3 (dominated by sigmoid argument rounding).
Env knobs: MHA_QK in {fp16 (default), f32r, fp32} picks the Q/K-path
matmul operand dtype (f32r/fp32 are slower, more accurate).
"""

import os
import numpy as np

import concourse.bacc as bacc
import concourse.mybir as mybir
import concourse.tile as tile
from concourse import bass_utils
from concourse.masks import make_identity

B, S, D, H = 2, 2048, 1024, 16
HD = 64
SCALE = float(D) ** 0.5
NCORES = 8
HPC = H // NCORES            # heads per core = 2
MS = HPC * HD                # per-core slice width = 128
P = 128
NCH = 8                      # phase-1 chunks (B * S/512)
DT16 = mybir.dt.float16
F32 = mybir.dt.float32

_QK = os.environ.get("MHA_QK", "fp16")
QK_DT = {"fp16": DT16, "f32r": mybir.dt.float32r, "fp32": F32}[_QK]


def build():
    nc = bacc.Bacc("TRN2", target_bir_lowering=False, debug=False)

    x_d = nc.dram_tensor("x", [B, S, D], F32, kind="ExternalInput").ap()
    wq_d = nc.dram_tensor("wq", [D, MS], F32, kind="ExternalInput").ap()
    wk_d = nc.dram_tensor("wk", [D, MS], F32, kind="ExternalInput").ap()
    wv_d = nc.dram_tensor("wv", [D, MS], F32, kind="ExternalInput").ap()
    wo_d = nc.dram_tensor("wo", [MS, D], F32, kind="ExternalInput").ap()
    out_d = nc.dram_tensor("out", [B, S, D], F32, kind="ExternalOutput").ap()

    qk16 = QK_DT == DT16

    with tile.TileContext(nc) as tc:
        with tc.tile_pool(name="persist", bufs=1) as pp:
            ident16 = pp.tile([P, P], DT16, name="ident16")
            if qk16:
                make_identity(nc, ident16[:])
                identq = ident16
            else:
                identq = pp.tile([P, P], QK_DT, name="identq")
                ident32 = pp.tile([P, P], F32, name="ident32")
                make_identity(nc, ident32[:])
                nc.vector.tensor_copy(identq[:], ident32[:])
                nc.vector.tensor_copy(ident16[:], ident32[:])
            ones512 = pp.tile([1, 512], DT16)
            nc.vector.memset(ones512[:], 1.0)
            ones128 = pp.tile([P, 1], DT16)
            nc.vector.memset(ones128[:], 1.0)

            # weights: fp32 loads + on-chip cast (SWDGE cast-DMA is ~3x slower
            # and would stall the start of the x-load stream)
            w_sb = {}
            wstage = {}
            for name, dram in (("wq", wq_d), ("wk", wk_d), ("wv", wv_d)):
                stage = pp.tile([P, D // P, MS], F32, name=f"{name}_stage")
                nc.gpsimd.dma_start(stage[:], dram.rearrange("(t p) m -> p t m", p=P))
                if QK_DT != F32:
                    t = pp.tile([P, D // P, MS], QK_DT, name=f"{name}_sb")
                    nc.vector.tensor_copy(t[:], stage[:])
                else:
                    t = stage
                w_sb[name] = t
            wo32 = pp.tile([P, D], F32, name="wo32")
            nc.gpsimd.dma_start(wo32[:], wo_d)
            wo_sb = pp.tile([P, 2, 512], DT16)
            nc.vector.tensor_copy(
                wo_sb[:], wo32[:].rearrange("p (c n) -> p c n", c=2)
            )

            # big persistent tensors
            qsb = pp.tile([P, HPC, S], QK_DT)    # [(b,hd), head, qpos], b1 negated
            ksb = pp.tile([P, HPC, S], QK_DT)    # [(b,hd), head, kpos]
            vt_sb = pp.tile([P, B, S], DT16)     # [(h,hd), batch, kpos], b1 negated
            v_sb = pp.tile([P, S // P, HPC, B, HD], DT16)  # [k, ktile, h, b, hd]
            vals_sb = pp.tile([P, B, S], DT16)   # [(h,hd), batch, qpos]
            c1_sb = pp.tile([1, HPC, HD], DT16)  # +colsum(V1) per head

            # ---------------- phase 1: xT + Q/K/V projections ----------------
            with tc.tile_pool(name="p1sb", bufs=8) as p1sb, \
                 tc.tile_pool(name="p1xt", bufs=4) as p1xt, \
                 tc.tile_pool(name="ps1", bufs=3, space="PSUM") as ps1, \
                 tc.tile_pool(name="ps1v", bufs=2, space="PSUM") as ps1v:
                def emit_proj(c, xt):
                    # Q/K/V projections for chunk c from its xT tile
                    b, j = divmod(c, NCH // B)
                    for name, dest, neg in (("wq", qsb, True), ("wk", ksb, False)):
                        ps = ps1.tile([P, 512], F32, tag="proj", name="ps_p")
                        for t in range(D // P):
                            nc.tensor.matmul(
                                ps[:], w_sb[name][:, t, :], xt[:, t, :],
                                start=(t == 0), stop=(t == D // P - 1),
                            )
                        for h in range(HPC):
                            nc.vector.tensor_scalar_mul(
                                dest[b * HD:(b + 1) * HD, h, j * 512:(j + 1) * 512],
                                ps[h * HD:(h + 1) * HD, :],
                                -1.0 if (neg and b == 1) else 1.0,
                            )
                    ps = ps1.tile([P, 512], F32, tag="proj", name="ps_p")
                    for t in range(D // P):
                        nc.tensor.matmul(
                            ps[:], w_sb["wv"][:, t, :], xt[:, t, :],
                            start=(t == 0), stop=(t == D // P - 1),
                        )
                    nc.vector.tensor_scalar_mul(
                        vt_sb[:, b, j * 512:(j + 1) * 512], ps[:],
                        -1.0 if b == 1 else 1.0,
                    )
                    # V natural layout for the 4 k-tiles of this chunk
                    for blk in range(4):
                        t = j * 4 + blk
                        pvt = ps1v.tile([P, P], DT16, tag="vt", name="pvt")
                        nc.tensor.transpose(
                            pvt[:], vt_sb[:, b, t * P:(t + 1) * P], ident16[:]
                        )
                        nc.vector.tensor_copy(
                            v_sb[:, t, :, b, :],
                            pvt[:].rearrange("p (h d) -> p h d", h=HPC),
                        )

                # software-pipelined: projections lag one chunk behind the
                # transposes so PE never waits on the xT psum->sbuf copies
                prev_chunk = None
                for c in range(NCH):
                    b, j = divmod(c, NCH // B)
                    xt = p1xt.tile([P, D // P, 512], QK_DT, tag="xt")
                    for blk in range(4):
                        src = x_d[b, j * 512 + blk * P: j * 512 + (blk + 1) * P, :]
                        dma_eng = nc.sync if (c * 4 + blk) % 2 == 0 else nc.gpsimd
                        if qk16:
                            xb32 = p1sb.tile([P, D], F32, tag="xb32")
                            if c == 0:
                                # split the very first loads across both rings
                                # so the pipeline fills before the DMA stream
                                # reaches steady state
                                nc.sync.dma_start(xb32[:, :D // 2], src[:, :D // 2])
                                nc.gpsimd.dma_start(xb32[:, D // 2:], src[:, D // 2:])
                            else:
                                dma_eng.dma_start(xb32[:], src)
                            xb = p1sb.tile([P, D], DT16, tag="xb")
                            nc.scalar.copy(xb[:], xb32[:])
                        else:
                            xb = p1sb.tile([P, D], QK_DT, tag="xb")
                            dma_eng.dma_start(xb[:], src)
                        for g in range(2):
                            pt = ps1.tile([P, 512], QK_DT, tag="tpose")
                            for t4 in range(4):
                                dt_i = g * 4 + t4
                                nc.tensor.transpose(
                                    pt[:, t4 * P:(t4 + 1) * P],
                                    xb[:, dt_i * P:(dt_i + 1) * P],
                                    identq[:],
                                )
                            nc.vector.tensor_copy(
                                xt[:, g * 4:(g + 1) * 4, blk * P:(blk + 1) * P],
                                pt[:].rearrange("p (t n) -> p t n", t=4),
                            )
                    if prev_chunk is not None:
                        emit_proj(*prev_chunk)
                    prev_chunk = (c, xt)
                emit_proj(*prev_chunk)

            # ------------- phase 1b (Q proj) + 2 + 3, interleaved -------------
            with tc.tile_pool(name="p2a", bufs=8) as p2a, \
                 tc.tile_pool(name="p3o", bufs=3) as p3o, \
                 tc.tile_pool(name="ps2d", bufs=2, space="PSUM") as ps2d, \
                 tc.tile_pool(name="ps2av", bufs=2, space="PSUM") as ps2av, \
                 tc.tile_pool(name="ps3", bufs=2, space="PSUM") as ps3:
                def emit_c1():
                    # colsums of all (h, b) V columns; extract b=1 (negated)
                    pc1 = ps3.tile([1, HPC * B * HD], F32, tag="o", name="pc1")
                    for t in range(S // P):
                        nc.tensor.matmul(
                            pc1[:], ones128[:],
                            v_sb[:, t, :, :, :].rearrange("p h b d -> p (h b d)"),
                            start=(t == 0), stop=(t == S // P - 1),
                        )
                    nc.vector.tensor_scalar_mul(
                        c1_sb[:],
                        pc1[:].rearrange(
                            "p (h b d) -> p h b d", h=HPC, b=B)[:, :, 1, :],
                        -1.0,
                    )

                def emit_out_block(b, si, tail=False):
                    # one output-projection s-block (phase 3); in the trailing
                    # run (after the last sigmoid) ACT and the sync ring are
                    # idle, so spread the copies and DMAs across them
                    ot = p3o.tile([P, D], F32, tag="ot", name="ot")
                    for nch in range(2):
                        po = ps3.tile([P, 512], F32, tag="o", name="po")
                        nc.tensor.matmul(
                            po[:],
                            vals_sb[:, b, si * P:(si + 1) * P],
                            wo_sb[:, nch, :],
                            start=True, stop=True,
                        )
                        if tail and nch == 1:
                            nc.scalar.copy(ot[:, nch * 512:(nch + 1) * 512], po[:])
                        else:
                            nc.vector.tensor_copy(
                                ot[:, nch * 512:(nch + 1) * 512], po[:]
                            )
                    ring = nc.scalar if (tail and si % 2 == 0) else nc.sync
                    ring.dma_start(out_d[b, si * P:(si + 1) * P, :], ot[:])

                NTP = S // P // 2
                for qc in range(S // 512):
                    # attention, both heads, software-pipelined: AV lags one
                    # k-pair behind scores/sigmoid so PE never waits on ACT;
                    # out-proj blocks of the previous q-chunk fill PE slack.
                    pavs = {}
                    for h in range(HPC):
                        pavs[h] = ps2av.tile([P, 512], F32, tag="av", name=f"pav{h}")
                    prev_at = None
                    for tp in range(NTP):
                        ats = {}
                        for h in range(HPC):
                            pd = ps2d.tile([P, 1024], F32, tag="d", name="pd")
                            for u in range(2):
                                t = tp * 2 + u
                                nc.tensor.matmul(
                                    pd[:, u * 512:(u + 1) * 512],
                                    ksb[:, h, t * P:(t + 1) * P],
                                    qsb[:, h, qc * 512:(qc + 1) * 512],
                                    start=True, stop=True,
                                )
                            at = p2a.tile([P, 1024], DT16, tag="at", name="at")
                            nc.scalar.activation(
                                at[:], pd[:],
                                mybir.ActivationFunctionType.Sigmoid,
                                scale=1.0 / SCALE,
                            )
                            ats[h] = at
                        if prev_at is not None:
                            ptp, pats = prev_at
                            for h in range(HPC):
                                for u in range(2):
                                    t = ptp * 2 + u
                                    nc.tensor.matmul(
                                        pavs[h][:],
                                        v_sb[:, t, h, :, :].rearrange(
                                            "p b d -> p (b d)"),
                                        pats[h][:, u * 512:(u + 1) * 512],
                                        start=(t == 0), stop=False,
                                    )
                        if qc > 0 and tp < 8:
                            b, sq = divmod(tp, 4)
                            emit_out_block(b, (qc - 1) * 4 + sq)
                        elif qc == 0 and tp == 1:
                            emit_c1()
                        prev_at = (tp, ats)
                    ptp, pats = prev_at
                    for h in range(HPC):
                        for u in range(2):
                            t = ptp * 2 + u
                            nc.tensor.matmul(
                                pavs[h][:],
                                v_sb[:, t, h, :, :].rearrange("p b d -> p (b d)"),
                                pats[h][:, u * 512:(u + 1) * 512],
                                start=False, stop=False,
                            )
                        nc.tensor.matmul(
                            pavs[h][HD:2 * HD, :], c1_sb[:, h, :], ones512[:],
                            start=False, stop=True,
                        )
                        for b in range(B):
                            nc.vector.tensor_copy(
                                vals_sb[h * HD:(h + 1) * HD, b,
                                        qc * 512:(qc + 1) * 512],
                                pavs[h][b * HD:(b + 1) * HD, :],
                            )
                # trailing out-proj blocks for the last q-chunk
                for b in range(B):
                    for sq in range(4):
                        emit_out_block(b, (S // 512 - 1) * 4 + sq, tail=True)

    nc.compile()
    return nc


_NC = None


def _get_nc():
    global _NC
    if _NC is None:
        _NC = build()
    return _NC


def kernel(x, w_q, w_k, w_v, W_o, _trace=False):
    x = np.ascontiguousarray(np.asarray(x, dtype=np.float32))
    w_q = np.asarray(w_q, dtype=np.float32)
    w_k = np.asarray(w_k, dtype=np.float32)
    w_v = np.asarray(w_v, dtype=np.float32)
    W_o = np.asarray(W_o, dtype=np.float32)

    nc = _get_nc()
    in_maps = []
    for i in range(NCORES):
        cs = slice(i * MS, (i + 1) * MS)
        in_maps.append({
            "x": x,
            "wq": np.ascontiguousarray(w_q[:, cs]),
            "wk": np.ascontiguousarray(w_k[:, cs]),
            "wv": np.ascontiguousarray(w_v[:, cs]),
            "wo": np.ascontiguousarray(W_o[cs, :]),
        })
    try:
        res = bass_utils.run_bass_kernel_spmd(
            nc, in_maps, core_ids=list(range(NCORES)), trace=_trace
        )
    except Exception:
        # transient NRT exec failures have been observed to succeed on retry
        res = bass_utils.run_bass_kernel_spmd(
            nc, in_maps, core_ids=list(range(NCORES)), trace=_trace
        )
    out = res.results[0]["out"].astype(np.float32).copy()
    for i in range(1, NCORES):
        out += res.results[i]["out"]
    if _trace:
        return out, res
    return out


# revision 39
# speedup vs baseline: 1.0195x; 1.0195x over previous
"""Trainium2 Bass kernel for nn_MHA_65429531787938.

MHA with a faithful-quirk softmax over dim=0 (the batch axis, B=2).
For B=2 the batch-softmax collapses to an elementwise sigmoid:
    attn0 = sigmoid((s0 - s1)/SCALE),  attn1 = 1 - attn0
and (1-A0) @ V1 = colsum(V1) - A0 @ V1, so a single attention matrix
serves both batches.

Sharding: tensor-parallel over the 16 heads -> 2 heads per core
(columns of w_q/w_k/w_v, rows of W_o). Each core consumes the full x
and produces a partial output (its heads' contribution to out = vals @ W_o);
the host sums the 8 partials.

Per-core pipeline (heads h0=2i, h1=2i+1 -> a 128-wide slice of q/k/v dims):
  phase 1: x cast to fp16 on load -> xT via PE transpose; qT,kT,vT
           projections (N=512 fp16 matmuls, fp32 psum);
           qT/kT stored batch-stacked per head ([Q0;-Q1] / [K0;K1]);
           vT -> V natural via PE transpose (V1 stored negated).
  phase 2: d^T = K0@Q0^T - K1@Q1^T in one fused matmul (contraction=128);
           A0^T = sigmoid(d^T/SCALE) on ACT (fp16 out);
           psum_av = [V0 | -V1] @ A0^T + rank-1 colsum(V1) correction.
  phase 3: out_partial = vals @ W_o_slice (fp16 operands, fp32 psum).

Precision: fp16 operands everywhere, fp32 accumulation; measured
end-to-end rel err ~3e-3 (dominated by sigmoid argument rounding).
Env knobs: MHA_QK in {fp16 (default), f32r, fp32} picks the Q/K-path
matmul operand dtype (f32r/fp32 are slower, more accurate).
"""

import os
import numpy as np

import concourse.bacc as bacc
import concourse.mybir as mybir
import concourse.tile as tile
from concourse import bass_utils
from concourse.masks import make_identity

B, S, D, H = 2, 2048, 1024, 16
HD = 64
SCALE = float(D) ** 0.5
NCORES = 8
HPC = H // NCORES            # heads per core = 2
MS = HPC * HD                # per-core slice width = 128
P = 128
NCH = 8                      # phase-1 chunks (B * S/512)
DT16 = mybir.dt.float16
F32 = mybir.dt.float32

_QK = os.environ.get("MHA_QK", "fp16")
QK_DT = {"fp16": DT16, "f32r": mybir.dt.float32r, "fp32": F32}[_QK]


def build():
    nc = bacc.Bacc("TRN2", target_bir_lowering=False, debug=False)

    x_d = nc.dram_tensor("x", [B, S, D], F32, kind="ExternalInput").ap()
    wq_d = nc.dram_tensor("wq", [D, MS], F32, kind="ExternalInput").ap()
    wk_d = nc.dram_tensor("wk", [D, MS], F32, kind="ExternalInput").ap()
    wv_d = nc.dram_tensor("wv", [D, MS], F32, kind="ExternalInput").ap()
    wo_d = nc.dram_tensor("wo", [MS, D], F32, kind="ExternalInput").ap()
    out_d = nc.dram_tensor("out", [B, S, D], F32, kind="ExternalOutput").ap()

    qk16 = QK_DT == DT16

    with tile.TileContext(nc) as tc:
        with tc.tile_pool(name="persist", bufs=1) as pp:
            ident16 = pp.tile([P, P], DT16, name="ident16")
            if qk16:
                make_identity(nc, ident16[:])
                identq = ident16
            else:
                identq = pp.tile([P, P], QK_DT, name="identq")
                ident32 = pp.tile([P, P], F32, name="ident32")
                make_identity(nc, ident32[:])
                nc.vector.tensor_copy(identq[:], ident32[:])
                nc.vector.tensor_copy(ident16[:], ident32[:])
            ones512 = pp.tile([1, 512], DT16)
            nc.vector.memset(ones512[:], 1.0)
            ones128 = pp.tile([P, 1], DT16)
            nc.vector.memset(ones128[:], 1.0)

            # weights: fp32 loads + on-chip cast (SWDGE cast-DMA is ~3x slower
            # and would stall the start of the x-load stream)
            w_sb = {}
            wstage = {}
            for name, dram in (("wq", wq_d), ("wk", wk_d), ("wv", wv_d)):
                stage = pp.tile([P, D // P, MS], F32, name=f"{name}_stage")
                nc.gpsimd.dma_start(stage[:], dram.rearrange("(t p) m -> p t m", p=P))
                if QK_DT != F32:
                    t = pp.tile([P, D // P, MS], QK_DT, name=f"{name}_sb")
                    nc.vector.tensor_copy(t[:], stage[:])
                else:
                    t = stage
                w_sb[name] = t
            wo32 = pp.tile([P, D], F32, name="wo32")
            nc.gpsimd.dma_start(wo32[:], wo_d)
            wo_sb = pp.tile([P, 2, 512], DT16)
            nc.vector.tensor_copy(
                wo_sb[:], wo32[:].rearrange("p (c n) -> p c n", c=2)
            )

            # big persistent tensors
            qsb = pp.tile([P, HPC, S], QK_DT)    # [(b,hd), head, qpos], b1 negated
            ksb = pp.tile([P, HPC, S], QK_DT)    # [(b,hd), head, kpos]
            vt_sb = pp.tile([P, B, S], DT16)     # [(h,hd), batch, kpos], b1 negated
            v_sb = pp.tile([P, S // P, HPC, B, HD], DT16)  # [k, ktile, h, b, hd]
            vals_sb = pp.tile([P, B, S], DT16)   # [(h,hd), batch, qpos]
            c1_sb = pp.tile([1, HPC, HD], DT16)  # +colsum(V1) per head

            # ---------------- phase 1: xT + Q/K/V projections ----------------
            with tc.tile_pool(name="p1sb", bufs=8) as p1sb, \
                 tc.tile_pool(name="p1xt", bufs=4) as p1xt, \
                 tc.tile_pool(name="ps1", bufs=3, space="PSUM") as ps1, \
                 tc.tile_pool(name="ps1v", bufs=2, space="PSUM") as ps1v:
                def emit_proj(c, xt):
                    # Q/K/V projections for chunk c from its xT tile
                    b, j = divmod(c, NCH // B)
                    for name, dest, neg in (("wq", qsb, True), ("wk", ksb, False)):
                        ps = ps1.tile([P, 512], F32, tag="proj", name="ps_p")
                        for t in range(D // P):
                            nc.tensor.matmul(
                                ps[:], w_sb[name][:, t, :], xt[:, t, :],
                                start=(t == 0), stop=(t == D // P - 1),
                            )
                        for h in range(HPC):
                            nc.vector.tensor_scalar_mul(
                                dest[b * HD:(b + 1) * HD, h, j * 512:(j + 1) * 512],
                                ps[h * HD:(h + 1) * HD, :],
                                -1.0 if (neg and b == 1) else 1.0,
                            )
                    ps = ps1.tile([P, 512], F32, tag="proj", name="ps_p")
                    for t in range(D // P):
                        nc.tensor.matmul(
                            ps[:], w_sb["wv"][:, t, :], xt[:, t, :],
                            start=(t == 0), stop=(t == D // P - 1),
                        )
                    nc.vector.tensor_scalar_mul(
                        vt_sb[:, b, j * 512:(j + 1) * 512], ps[:],
                        -1.0 if b == 1 else 1.0,
                    )
                    # V natural layout for the 4 k-tiles of this chunk
                    for blk in range(4):
                        t = j * 4 + blk
                        pvt = ps1v.tile([P, P], DT16, tag="vt", name="pvt")
                        nc.tensor.transpose(
                            pvt[:], vt_sb[:, b, t * P:(t + 1) * P], ident16[:]
                        )
                        nc.vector.tensor_copy(
                            v_sb[:, t, :, b, :],
                            pvt[:].rearrange("p (h d) -> p h d", h=HPC),
                        )

                # software-pipelined: projections lag one chunk behind the
                # transposes so PE never waits on the xT psum->sbuf copies
                prev_chunk = None
                for c in range(NCH):
                    b, j = divmod(c, NCH // B)
                    xt = p1xt.tile([P, D // P, 512], QK_DT, tag="xt")
                    for blk in range(4):
                        src = x_d[b, j * 512 + blk * P: j * 512 + (blk + 1) * P, :]
                        dma_eng = nc.sync if (c * 4 + blk) % 2 == 0 else nc.gpsimd
                        if qk16:
                            xb32 = p1sb.tile([P, D], F32, tag="xb32")
                            if c == 0:
                                # split the very first loads across both rings
                                # so the pipeline fills before the DMA stream
                                # reaches steady state
                                nc.sync.dma_start(xb32[:, :D // 2], src[:, :D // 2])
                                nc.gpsimd.dma_start(xb32[:, D // 2:], src[:, D // 2:])
                            else:
                                dma_eng.dma_start(xb32[:], src)
                            xb = p1sb.tile([P, D], DT16, tag="xb")
                            nc.scalar.copy(xb[:], xb32[:])
                        else:
                            xb = p1sb.tile([P, D], QK_DT, tag="xb")
                            dma_eng.dma_start(xb[:], src)
                        for g in range(2):
                            pt = ps1.tile([P, 512], QK_DT, tag="tpose")
                            for t4 in range(4):
                                dt_i = g * 4 + t4
                                nc.tensor.transpose(
                                    pt[:, t4 * P:(t4 + 1) * P],
                                    xb[:, dt_i * P:(dt_i + 1) * P],
                                    identq[:],
                                )
                            nc.vector.tensor_copy(
                                xt[:, g * 4:(g + 1) * 4, blk * P:(blk + 1) * P],
                                pt[:].rearrange("p (t n) -> p t n", t=4),
                            )
                    if prev_chunk is not None:
                        emit_proj(*prev_chunk)
                    prev_chunk = (c, xt)
                emit_proj(*prev_chunk)

            # ------------- phase 1b (Q proj) + 2 + 3, interleaved -------------
            with tc.tile_pool(name="p2a", bufs=8) as p2a, \
                 tc.tile_pool(name="p3o", bufs=3) as p3o, \
                 tc.tile_pool(name="ps2d", bufs=2, space="PSUM") as ps2d, \
                 tc.tile_pool(name="ps2av", bufs=2, space="PSUM") as ps2av, \
                 tc.tile_pool(name="ps3", bufs=2, space="PSUM") as ps3:
                # colsums of all (h, b) V columns; extract b=1 (stored negated)
                pc1 = ps3.tile([1, HPC * B * HD], F32, tag="o")
                for t in range(S // P):
                    nc.tensor.matmul(
                        pc1[:], ones128[:],
                        v_sb[:, t, :, :, :].rearrange("p h b d -> p (h b d)"),
                        start=(t == 0), stop=(t == S // P - 1),
                    )
                nc.vector.tensor_scalar_mul(
                    c1_sb[:],
                    pc1[:].rearrange("p (h b d) -> p h b d", h=HPC, b=B)[:, :, 1, :],
                    -1.0,
                )

                def emit_out_block(b, si, tail=False):
                    # one output-projection s-block (phase 3); in the trailing
                    # run (after the last sigmoid) ACT and the sync ring are
                    # idle, so spread the copies and DMAs across them
                    ot = p3o.tile([P, D], F32, tag="ot", name="ot")
                    for nch in range(2):
                        po = ps3.tile([P, 512], F32, tag="o", name="po")
                        nc.tensor.matmul(
                            po[:],
                            vals_sb[:, b, si * P:(si + 1) * P],
                            wo_sb[:, nch, :],
                            start=True, stop=True,
                        )
                        if tail and nch == 1:
                            nc.scalar.copy(ot[:, nch * 512:(nch + 1) * 512], po[:])
                        else:
                            nc.vector.tensor_copy(
                                ot[:, nch * 512:(nch + 1) * 512], po[:]
                            )
                    ring = nc.scalar if (tail and si % 2 == 0) else nc.sync
                    ring.dma_start(out_d[b, si * P:(si + 1) * P, :], ot[:])

                NTP = S // P // 2
                for qc in range(S // 512):
                    # attention, both heads, software-pipelined: AV lags one
                    # k-pair behind scores/sigmoid so PE never waits on ACT;
                    # out-proj blocks of the previous q-chunk fill PE slack.
                    pavs = {}
                    for h in range(HPC):
                        pavs[h] = ps2av.tile([P, 512], F32, tag="av", name=f"pav{h}")
                    prev_at = None
                    for tp in range(NTP):
                        ats = {}
                        for h in range(HPC):
                            pd = ps2d.tile([P, 1024], F32, tag="d", name="pd")
                            for u in range(2):
                                t = tp * 2 + u
                                nc.tensor.matmul(
                                    pd[:, u * 512:(u + 1) * 512],
                                    ksb[:, h, t * P:(t + 1) * P],
                                    qsb[:, h, qc * 512:(qc + 1) * 512],
                                    start=True, stop=True,
                                )
                            at = p2a.tile([P, 1024], DT16, tag="at", name="at")
                            nc.scalar.activation(
                                at[:], pd[:],
                                mybir.ActivationFunctionType.Sigmoid,
                                scale=1.0 / SCALE,
                            )
                            ats[h] = at
                        if prev_at is not None:
                            ptp, pats = prev_at
                            for h in range(HPC):
                                for u in range(2):
                                    t = ptp * 2 + u
                                    nc.tensor.matmul(
                                        pavs[h][:],
                                        v_sb[:, t, h, :, :].rearrange(
                                            "p b d -> p (b d)"),
                                        pats[h][:, u * 512:(u + 1) * 512],
                                        start=(t == 0), stop=False,
                                    )
                        if qc > 0 and tp < 8:
                            b, sq = divmod(tp, 4)
                            emit_out_block(b, (qc - 1) * 4 + sq)
                        prev_at = (tp, ats)
                    ptp, pats = prev_at
                    for h in range(HPC):
                        for u in range(2):
                            t = ptp * 2 + u
                            nc.tensor.matmul(
                                pavs[h][:],
                                v_sb[:, t, h, :, :].rearrange("p b d -> p (b d)"),
                                pats[h][:, u * 512:(u + 1) * 512],
                                start=False, stop=False,
                            )
                        nc.tensor.matmul(
                            pavs[h][HD:2 * HD, :], c1_sb[:, h, :], ones512[:],
                            start=False, stop=True,
                        )
                        for b in range(B):
                            nc.vector.tensor_copy(
                                vals_sb[h * HD:(h + 1) * HD, b,
                                        qc * 512:(qc + 1) * 512],
                                pavs[h][b * HD:(b + 1) * HD, :],
                            )
                # trailing out-proj blocks for the last q-chunk
                for b in range(B):
                    for sq in range(4):
                        emit_out_block(b, (S // 512 - 1) * 4 + sq, tail=True)

    nc.compile()
    return nc


_NC = None


def _get_nc():
    global _NC
    if _NC is None:
        _NC = build()
    return _NC


def kernel(x, w_q, w_k, w_v, W_o, _trace=False):
    x = np.ascontiguousarray(np.asarray(x, dtype=np.float32))
    w_q = np.asarray(w_q, dtype=np.float32)
    w_k = np.asarray(w_k, dtype=np.float32)
    w_v = np.asarray(w_v, dtype=np.float32)
    W_o = np.asarray(W_o, dtype=np.float32)

    nc = _get_nc()
    in_maps = []
    for i in range(NCORES):
        cs = slice(i * MS, (i + 1) * MS)
        in_maps.append({
            "x": x,
            "wq": np.ascontiguousarray(w_q[:, cs]),
            "wk": np.ascontiguousarray(w_k[:, cs]),
            "wv": np.ascontiguousarray(w_v[:, cs]),
            "wo": np.ascontiguousarray(W_o[cs, :]),
        })
    try:
        res = bass_utils.run_bass_kernel_spmd(
            nc, in_maps, core_ids=list(range(NCORES)), trace=_trace
        )
    except Exception:
        # transient NRT exec failures have been observed to succeed on retry
        res = bass_utils.run_bass_kernel_spmd(
            nc, in_maps, core_ids=list(range(NCORES)), trace=_trace
        )
    out = res.results[0]["out"].astype(np.float32).copy()
    for i in range(1, NCORES):
        out += res.results[i]["out"]
    if _trace:
        return out, res
    return out
